# revision 4
# baseline (speedup 1.0000x reference)
"""Trainium2 Bass kernel for top-1 MoE routing + expert FFN — v2.

Changes vs v1 (baseline):
  gate: host pre-transposes x (placement-only), so the gate launch does NO
    PE transposes / PSUM->SBUF copies; logits come straight from fp32
    matmuls with xT chunks as the stationary operand. The bf16 copy of x
    for the FFN is done on host (dtype conversion, not model math).
  ffn: w1/w2 pre-converted to bf16 and pre-tiled on host (halves weight
    DMA, removes all DVE conversion work); gelu is a single ACT
    instruction (Gelu_apprx_tanh) reading PSUM and writing bf16 hT;
    output scaling moved to the ACT engine (Copy with per-partition
    scale), so DVE does nothing in the FFN launch.
  route: constants (ut/eid) hoisted out of the rep body.

All routing DECISIONS (logits, argmax, softmax value, cumsum positions,
capacity mask) remain on device. Host does placement only: transpose,
gather, scatter, dtype conversion.
"""

import functools

import ml_dtypes
import numpy as np

import concourse.bacc as bacc
import concourse.mybir as mybir
import concourse.tile as tile
from concourse.bass_utils import run_bass_kernel_spmd

F32 = mybir.dt.float32
BF16 = mybir.dt.bfloat16
U32 = mybir.dt.uint32

N_CORES = 8


# --------------------------------------------------------------------------
# Launch 1: gate (token-parallel)
# --------------------------------------------------------------------------
def build_gate_nc(S, D, E, n_cores=N_CORES, reps=1):
    """Per core: xT_sl [T, 128, DC*128] f32 (host-transposed token slice;
    xT_sl[t, p, dc*128+s] = x[t*128+s, dc*128+p]) -> e_out [128, T] u32,
    g_out [128, T] f32.  (T = Sl/128 tiles; token s_local = t*128 + p.)"""
    Sl = S // n_cores
    T = Sl // 128
    DC = D // 128
    nc = bacc.Bacc("TRN2", target_bir_lowering=False, debug=False)
    xT_sl = nc.dram_tensor("xT_sl", [T, 128, DC * 128], F32, kind="ExternalInput")
    wg = nc.dram_tensor("wg", [D, E], F32, kind="ExternalInput")
    e_out = nc.dram_tensor("e_out", [128, T], U32, kind="ExternalOutput")
    g_out = nc.dram_tensor("g_out", [128, T], F32, kind="ExternalOutput")

    with tile.TileContext(nc) as tc:
        with (
            tc.tile_pool(name="const", bufs=1) as constp,
            tc.tile_pool(name="xt", bufs=3) as xtp,
            tc.tile_pool(name="small", bufs=4) as small,
            tc.tile_pool(name="res", bufs=1) as resp,
            tc.tile_pool(name="psl", bufs=2, space="PSUM") as psl,
        ):
            wg_sb = constp.tile([128, DC, E], F32)
            nc.sync.dma_start(wg_sb[:], wg.ap().rearrange("(c p) e -> p c e", p=128))
            e_sb = resp.tile([128, T], U32)
            g_sb = resp.tile([128, T], F32)

            for t in [t for _ in range(reps) for t in range(T)]:
                xt_sb = xtp.tile([128, DC, 128], F32)
                nc.sync.dma_start(
                    xt_sb[:], xT_sl.ap()[t].rearrange("p (a b) -> p a b", a=DC)
                )
                # logits [128 s, E] accumulated over d-chunks
                pl = psl.tile([128, E], F32)
                for d in range(DC):
                    nc.tensor.matmul(
                        pl[:],
                        lhsT=xt_sb[:, d, :],
                        rhs=wg_sb[:, d, :],
                        start=(d == 0),
                        stop=(d == DC - 1),
                    )
                lsb = small.tile([128, E], F32)
                nc.vector.tensor_copy(lsb[:], pl[:])
                # g = 1 / sum(exp(l - max))  (softmax value at the argmax)
                mx = small.tile([128, 1], F32)
                nc.vector.tensor_reduce(
                    mx[:], lsb[:], axis=mybir.AxisListType.X, op=mybir.AluOpType.max
                )
                nmx = small.tile([128, 1], F32)
                nc.vector.tensor_scalar_mul(nmx[:], mx[:], -1.0)
                ex = small.tile([128, E], F32)
                den = small.tile([128, 1], F32)
                nc.scalar.activation(
                    ex[:],
                    lsb[:],
                    mybir.ActivationFunctionType.Exp,
                    bias=nmx[:],
                    scale=1.0,
                    accum_out=den[:],
                )
                nc.vector.reciprocal(g_sb[:, t : t + 1], den[:])
                # argmax over experts (first max wins, like jnp.argmax)
                top8 = small.tile([128, 8], F32)
                nc.vector.max(top8[:], lsb[:])
                midx = small.tile([128, 8], U32)
                nc.vector.max_index(midx[:], top8[:], lsb[:])
                nc.vector.tensor_copy(e_sb[:, t : t + 1], midx[:, 0:1])

            nc.sync.dma_start(e_out.ap()[:], e_sb[:])
            nc.sync.dma_start(g_out.ap()[:], g_sb[:])
    nc.compile()
    return nc


# --------------------------------------------------------------------------
# Launch 2: routing positions (expert-parallel)  [unchanged math from v1]
# --------------------------------------------------------------------------
def build_route_nc(S, E, reps=1):
    """Per core (expert k): exclusive position of each token within expert k
    plus the capacity keep-mask.

    Inputs : ef [128, J] f32 (expert id per token; s = j*128 + p),
             eid [128, 1] f32, ut [128, 128] f32 (ut[q, p] = 1 if q < p:
             STRICT upper -> exclusive within-column prefix directly).
    Outputs: pos_out [128, J] f32, valid_out [128, J] f32.

    pos = (ut^T @ m) + broadcast(excl-scan(colsum(m))) accumulated in one
    PSUM bank; valid = (pos < C) * m.
    """
    J = S // 128
    C = S // E
    nc = bacc.Bacc("TRN2", target_bir_lowering=False, debug=False)
    ef = nc.dram_tensor("ef", [128, J], F32, kind="ExternalInput")
    eid = nc.dram_tensor("eid", [128, 1], F32, kind="ExternalInput")
    ut = nc.dram_tensor("ut", [128, 128], F32, kind="ExternalInput")
    pos_out = nc.dram_tensor("pos_out", [128, J], F32, kind="ExternalOutput")
    valid_out = nc.dram_tensor("valid_out", [128, J], F32, kind="ExternalOutput")

    with tile.TileContext(nc) as tc:
        with (
            tc.tile_pool(name="c", bufs=1) as cp,
        ):
            eid_sb = cp.tile([128, 1], F32)
            nc.sync.dma_start(eid_sb[:], eid.ap())
            ut_sb = cp.tile([128, 128], F32)
            nc.sync.dma_start(ut_sb[:], ut.ap())
            ones_sb = cp.tile([1, 128], F32)
            nc.vector.memset(ones_sb[:], 1.0)
            ones_col = cp.tile([128, 1], F32)
            nc.vector.memset(ones_col[:], 1.0)
            zrow = cp.tile([1, J], F32)
            nc.vector.memset(zrow[:], 0.0)

            for _rep in range(reps):
                with (
                    tc.tile_pool(name="p", bufs=1) as p,
                    tc.tile_pool(name="ps", bufs=1, space="PSUM") as ps,
                ):
                    ef_sb = p.tile([128, J], F32)
                    nc.sync.dma_start(ef_sb[:], ef.ap())
                    m_sb = p.tile([128, J], F32)
                    nc.vector.tensor_scalar(
                        m_sb[:],
                        ef_sb[:],
                        eid_sb[:],
                        None,
                        op0=mybir.AluOpType.is_equal,
                    )
                    # column totals (rank-1 matmul) -> exclusive prefix across
                    # columns (tokens are ordered column-major: s = j*128 + p)
                    ps_tot = ps.tile([1, J], F32)
                    nc.tensor.matmul(
                        ps_tot[:], lhsT=ones_col[:], rhs=m_sb[:], start=True, stop=True
                    )
                    sc = p.tile([1, J], F32)
                    nc.vector.tensor_tensor_scan(
                        sc[:],
                        ps_tot[:],
                        zrow[:],
                        0.0,
                        op0=mybir.AluOpType.add,
                        op1=mybir.AluOpType.add,
                    )
                    off = p.tile([1, J], F32)
                    nc.vector.tensor_sub(off[:], sc[:], ps_tot[:])
                    # pos = within-column exclusive prefix + column offset,
                    # both accumulated into one PSUM bank
                    ps_pos = ps.tile([128, J], F32)
                    nc.tensor.matmul(
                        ps_pos[:], lhsT=ut_sb[:], rhs=m_sb[:], start=True, stop=False
                    )
                    nc.tensor.matmul(
                        ps_pos[:], lhsT=ones_sb[:], rhs=off[:], start=False, stop=True
                    )
                    # keep-mask: member and pos < capacity
                    v_sb = p.tile([128, J], F32)
                    nc.vector.tensor_scalar(
                        v_sb[:], ps_pos[:], float(C), None, op0=mybir.AluOpType.is_lt
                    )
                    nc.vector.tensor_mul(v_sb[:], v_sb[:], m_sb[:])
                    pos_sb = p.tile([128, J], F32)
                    nc.vector.tensor_copy(pos_sb[:], ps_pos[:])
                    nc.sync.dma_start(pos_out.ap()[:], pos_sb[:])
                    nc.sync.dma_start(valid_out.ap()[:], v_sb[:])
    nc.compile()
    return nc


# --------------------------------------------------------------------------
# Launch 3: expert FFN (expert-parallel)
# --------------------------------------------------------------------------
def build_ffn_nc(S, D, E, F, reps=1):
    """Per core (expert k): gathered tokens -> gelu MLP -> scaled compact out.

    Inputs : xgT [128, DC*C] bf16 (xgT[p, d*C + c] = xg[c, d*128 + p]),
             gcol [128, CC] f32 (gcol[p, cc] = gate of slot cc*128 + p),
             w1h [FC, 128, DC*128] bf16 (w1h[fc, p, dc*128+fi] =
                 w1[dc*128+p, fc*128+fi]),
             w2h [NDB, FC, 128, DB] bf16 (w2h[db, fc, p, dj] =
                 w2[fc*128+p, db*DB+dj]).
    Outputs: out_c [C, D] f32 (slot-major, already gate-scaled).
    """
    C = S // E
    CC = C // 128
    DC = D // 128
    FC = F // 128
    DB = 512
    NDB = D // DB

    nc = bacc.Bacc("TRN2", target_bir_lowering=False, debug=False)
    xgT_in = nc.dram_tensor("xgT", [128, DC * C], BF16, kind="ExternalInput")
    gcol_in = nc.dram_tensor("gcol", [128, CC], F32, kind="ExternalInput")
    w1h = nc.dram_tensor("w1h", [FC, 128, DC * 128], BF16, kind="ExternalInput")
    w2h = nc.dram_tensor("w2h", [NDB, FC, 128, DB], BF16, kind="ExternalInput")
    out_c = nc.dram_tensor("out_c", [C, D], F32, kind="ExternalOutput")

    with tile.TileContext(nc) as tc:
      for _rep in range(reps):
        with (
            tc.tile_pool(name="hTp", bufs=1) as hTp,
            tc.tile_pool(name="ggp", bufs=1) as ggp,
        ):
            hT = hTp.tile([128, FC, C], BF16)
            gcol = ggp.tile([128, CC], F32)
            nc.sync.dma_start(gcol[:], gcol_in.ap())
            # ---- mm1: hT[f, c] = gelu(sum_d w1[d, f] xgT[d, c]) ----
            with (
                tc.tile_pool(name="xgp", bufs=1) as xgp,
                tc.tile_pool(name="w1p", bufs=3) as w1p,
                tc.tile_pool(name="ps1", bufs=2, space="PSUM") as ps1,
            ):
                xgT = xgp.tile([128, DC, C], BF16)
                xg_src = xgT_in.ap().rearrange("p (a b) -> p a b", a=DC)
                # issue xgT chunk 0 and the first w1 tile ahead of the rest:
                # the f=0 iteration below runs d-outer so its matmuls consume
                # chunks in DMA arrival order instead of waiting for the full
                # xgT load
                nc.sync.dma_start(xgT[:, 0, :], xg_src[:, 0, :])
                w1t0 = w1p.tile([128, DC, 128], BF16)
                nc.sync.dma_start(
                    w1t0[:], w1h.ap()[0].rearrange("p (a b) -> p a b", a=DC)
                )
                for d in range(1, DC):
                    nc.sync.dma_start(xgT[:, d, :], xg_src[:, d, :])
                for f in range(FC):
                    if f == 0:
                        w1t = w1t0
                    else:
                        w1t = w1p.tile([128, DC, 128], BF16)
                        nc.sync.dma_start(
                            w1t[:], w1h.ap()[f].rearrange("p (a b) -> p a b", a=DC)
                        )
                    ph = ps1.tile([128, C], F32)
                    if f == 0:
                        for d in range(DC):
                            for h in range(0, C, 512):
                                nc.tensor.matmul(
                                    ph[:, h : h + 512],
                                    lhsT=w1t[:, d, :],
                                    rhs=xgT[:, d, h : h + 512],
                                    start=(d == 0),
                                    stop=(d == DC - 1),
                                )
                    else:
                        for h in range(0, C, 512):
                            for d in range(DC):
                                nc.tensor.matmul(
                                    ph[:, h : h + 512],
                                    lhsT=w1t[:, d, :],
                                    rhs=xgT[:, d, h : h + 512],
                                    start=(d == 0),
                                    stop=(d == DC - 1),
                                )
                    nc.scalar.activation(
                        hT[:, f, :],
                        ph[:],
                        mybir.ActivationFunctionType.Gelu_apprx_tanh,
                    )

            # ---- mm2: out[c, d] = g[c] * sum_f hT[f, c] w2[f, d] ----
            with (
                tc.tile_pool(name="w2p", bufs=3) as w2p,
                tc.tile_pool(name="outp", bufs=4) as outp,
                tc.tile_pool(name="ps2", bufs=1, space="PSUM") as ps2,
            ):
                for db in range(NDB):
                    pso = [
                        ps2.tile([128, DB], F32, name=f"pso{c}", tag=f"pso{c}")
                        for c in range(CC)
                    ]
                    for f in range(FC):
                        w2t = w2p.tile([128, DB], BF16)
                        nc.sync.dma_start(w2t[:], w2h.ap()[db, f])
                        for c in range(CC):
                            nc.tensor.matmul(
                                pso[c][:],
                                lhsT=hT[:, f, c * 128 : (c + 1) * 128],
                                rhs=w2t[:],
                                start=(f == 0),
                                stop=(f == FC - 1),
                            )
                    for c in range(CC):
                        ob = outp.tile([128, DB], F32)
                        # alternate drain engines so the per-db drain tail
                        # halves (ACT and DVE run in parallel; both compute
                        # pso*g in f32 with identical rounding)
                        if c % 2 == 0:
                            nc.scalar.activation(
                                ob[:],
                                pso[c][:],
                                mybir.ActivationFunctionType.Copy,
                                scale=gcol[:, c : c + 1],
                            )
                        else:
                            nc.vector.tensor_scalar(
                                ob[:],
                                pso[c][:],
                                gcol[:, c : c + 1],
                                None,
                                op0=mybir.AluOpType.mult,
                            )
                        nc.sync.dma_start(
                            out_c.ap()[
                                c * 128 : (c + 1) * 128, db * DB : (db + 1) * DB
                            ],
                            ob[:],
                        )
    nc.compile()
    return nc


# --------------------------------------------------------------------------
# Host orchestration
# --------------------------------------------------------------------------
@functools.lru_cache(maxsize=None)
def _compiled(S, D, E, F, n_cores):
    return (
        build_gate_nc(S, D, E, n_cores),
        build_route_nc(S, E),
        build_ffn_nc(S, D, E, F),
    )


def _run_spmd(nc, in_maps, **kw):
    res = run_bass_kernel_spmd(nc, in_maps, core_ids=list(range(len(in_maps))), **kw)
    return res.results


def moe_forward(hidden_states, w_gate, w1, w2, n_cores=N_CORES, run=_run_spmd):
    B, T, D = hidden_states.shape
    E = w_gate.shape[1]
    F = w1.shape[2]
    S = B * T
    C = S // E
    CC = C // 128
    DC = D // 128
    FC = F // 128
    DB = 512
    NDB = D // DB
    Sl = S // n_cores
    J = S // 128
    x = np.ascontiguousarray(hidden_states.reshape(S, D), dtype=np.float32)
    nc_gate, nc_route, nc_ffn = _compiled(S, D, E, F, n_cores)

    # ---- launch 1: gate ----
    wg = np.ascontiguousarray(w_gate, dtype=np.float32)
    in1 = []
    T_tiles = Sl // 128
    for k in range(n_cores):
        xs = x[k * Sl : (k + 1) * Sl]  # [Sl, D]
        # xT_sl[t, p, dc*128+s] = xs[t*128+s, dc*128+p]
        xT = np.ascontiguousarray(
            xs.reshape(T_tiles, 128, DC, 128).transpose(0, 3, 2, 1).reshape(
                T_tiles, 128, DC * 128
            )
        )
        in1.append({"xT_sl": xT, "wg": wg})
    res1 = run(nc_gate, in1)

    # per-core outputs concat: column j = (k, t) -> token s = j*128 + p
    ef = np.concatenate([r["e_out"] for r in res1], axis=1).astype(np.float32)
    gf = np.concatenate([r["g_out"] for r in res1], axis=1)  # [128, J] f32

    # host-side bf16 copy of tokens for the FFN gather (dtype conversion)
    xb_full = np.concatenate(
        [x.astype(ml_dtypes.bfloat16), np.zeros((1, D), dtype=ml_dtypes.bfloat16)]
    )  # [S+1, D]

    # ---- launch 2: routing positions ----
    ut = np.tril(np.ones((128, 128), dtype=np.float32), -1).T  # ut[q,p] = q < p
    in2 = [
        {"ef": ef, "eid": np.full((128, 1), float(k), np.float32), "ut": ut}
        for k in range(n_cores)
    ]
    res2 = run(nc_route, in2)

    # ---- host glue: build per-expert slot -> token index lists (placement) --
    s_val = (np.arange(J)[None, :] * 128 + np.arange(128)[:, None]).astype(
        np.int64
    )  # [128, J]
    g_vec = np.empty(S, dtype=np.float32)
    g_vec[s_val.reshape(-1)] = gf.reshape(-1)
    ids_all = []
    in3 = []
    for k in range(n_cores):
        pos = res2[k]["pos_out"]
        valid = res2[k]["valid_out"] > 0.5
        ids = np.full(C, S, dtype=np.int64)  # default -> zero row
        ids[pos[valid].astype(np.int64)] = s_val[valid]
        ids_all.append(ids)
        xg = xb_full[ids]  # [C, D] bf16
        xgT = np.ascontiguousarray(
            xg.T.reshape(DC, 128, C).transpose(1, 0, 2).reshape(128, DC * C)
        )
        g_slot = np.where(ids < S, g_vec[np.minimum(ids, S - 1)], 0.0).astype(
            np.float32
        )
        gcol = np.ascontiguousarray(g_slot.reshape(CC, 128).T)
        # w1h[fc, p, dc*128+fi] = w1[k][dc*128+p, fc*128+fi]
        # (partition p = d-within-chunk: the matmul contraction dim)
        w1h = np.ascontiguousarray(
            np.asarray(w1[k], dtype=np.float32)
            .astype(ml_dtypes.bfloat16)
            .reshape(DC, 128, FC, 128)
            .transpose(2, 1, 0, 3)
            .reshape(FC, 128, DC * 128)
        )
        # w2h[db, fc, p, dj] = w2[k][fc*128+p, db*DB+dj]
        w2h = np.ascontiguousarray(
            np.asarray(w2[k], dtype=np.float32)
            .astype(ml_dtypes.bfloat16)
            .reshape(FC, 128, NDB, DB)
            .transpose(2, 0, 1, 3)
        )
        in3.append({"xgT": xgT, "gcol": gcol, "w1h": w1h, "w2h": w2h})

    # ---- launch 3: FFN ----
    res3 = run(nc_ffn, in3)

    # ---- host scatter (placement only) ----
    out = np.zeros((S, D), dtype=np.float32)
    for k in range(n_cores):
        ids = ids_all[k]
        filled = ids < S
        out[ids[filled]] = res3[k]["out_c"][filled]
    return out.reshape(B, T, D)


def kernel(**inputs):
    hs = np.asarray(inputs["hidden_states"], dtype=np.float32)
    wg = np.asarray(inputs["w_gate"], dtype=np.float32)
    w1 = np.asarray(inputs["w1"], dtype=np.float32)
    w2 = np.asarray(inputs["w2"], dtype=np.float32)
    return moe_forward(hs, wg, w1, w2)
